# Initial kernel scaffold
#
"""Trainium2 Bass kernel for a 2-layer GCN encoder (PyG GCNConv semantics).

Math (per gcn_conv): out = D^-1/2 (A+I) D^-1/2 (x @ W) + b, with relu
between the two convs.

Strategy (8 NeuronCores, SPMD):
  * Layer 1 is computed as (A_hat @ x) @ W1 + b1 (associativity), so the
    edge aggregation runs directly on the input x, which is replicated into
    every core's DRAM for free -> layer 1 needs no communication.
  * Nodes (aggregation outputs) are sharded by destination: core c owns
    nodes [6250c, 6250(c+1)). Edges are partitioned by dst owner and
    grouped by 128-node dst blocks.
  * Aggregation = gather + scatter-matmul: source rows are fetched with the
    GPSIMD dma_gather custom op (bf16 rows); a per-chunk selection matrix
    S[e, slot] = norm_e * (slot == dstoff_e) is built with one DVE
    tensor_scalar (iota compare), and TensorE matmuls with lhsT=S
    scatter-add 128-edge chunks into a [slot, feat] PSUM block.
  * Layer-1 aggregation lands node-major; a bf16 DMA-transpose (XBAR)
    produces the feature-major operand for the W1 GEMM. relu/bias run in
    the PSUM->SBUF epilogues. h2 = relu(out1) @ W2 stays local.
  * The only communication is an AllGather of h2, split into two
    half-shard collectives so layer-2 gathers of the first half overlap
    the second collective.
  * dma_gather indices are int16; tables are stored/addressed in two
    halves ordered by (local-half, owner, offset), so indices stay < 25088.
    Gather-call padding uses idx=-1 (descriptors skipped); per-core valid
    counts feed num_idxs_reg via a register.

Host-side work is limited to index/partition metadata (edge bucketing,
padding, degree-based norm coefficients) and dtype/layout staging.
"""
import sys
from collections import namedtuple
from contextlib import ExitStack

sys.path.insert(0, "/opt/trn_rl_repo")

import numpy as np
import ml_dtypes

import concourse.bacc as bacc
import concourse.mybir as mybir
import concourse.tile as tile
from concourse.bass_utils import run_bass_kernel_spmd

BF16 = ml_dtypes.bfloat16

Cfg = namedtuple("Cfg", "n_nodes in_ch hid out_ch ncores split")
DEFAULT_CFG = Cfg(50000, 512, 512, 256, 8, 0)

SUBCALL = 7          # max gather chunks per dma_gather call (SWDGE ring)


def _derived(cfg):
    npc = cfg.n_nodes // cfg.ncores
    nblk = (npc + 127) // 128
    last_rows = npc - 128 * (nblk - 1)
    npc2 = npc // 2
    return npc, nblk, last_rows, npc2


def _call_schedule(g_sizes, nblk):
    calls = []
    cgb = 0
    for b in range(nblk):
        for h in (0, 1):
            G = int(g_sizes[b, h])
            K = G // 128
            k0 = 0
            while k0 < K:
                kw = min(SUBCALL, K - k0)
                calls.append((cgb + k0, kw))
                k0 += kw
            cgb += K
    return calls


# ----------------------------------------------------------------- host prep

def _preprocess(x, edge_index, W1, b1, W2, b2, cfg=DEFAULT_CFG):
    x = np.asarray(x, dtype=np.float32)
    ei = np.asarray(edge_index)
    W1 = np.asarray(W1, dtype=np.float32)
    b1 = np.asarray(b1, dtype=np.float32)
    W2 = np.asarray(W2, dtype=np.float32)
    b2 = np.asarray(b2, dtype=np.float32)

    NPC, NBLK, LAST_ROWS, NPC2 = _derived(cfg)
    NCORES = cfg.ncores
    KG = cfg.hid // 128
    OUT_CH = cfg.out_ch
    n = x.shape[0]
    loops = np.arange(n, dtype=np.int64)
    src = np.concatenate([ei[0].astype(np.int64), loops])
    dst = np.concatenate([ei[1].astype(np.int64), loops])

    # degree (with self loops) and symmetric normalization
    deg = np.bincount(dst, minlength=n).astype(np.float32)
    dinv = np.where(deg > 0, 1.0 / np.sqrt(deg), 0.0).astype(np.float32)
    norm = dinv[src] * dinv[dst]

    owner = dst // NPC
    block = (dst % NPC) // 128
    dstoff = (dst % NPC) % 128
    # source table coordinates: (half, owner, offset) ordering
    s_loc = src % NPC
    half = (s_loc >= NPC2).astype(np.int64)
    lidx = (src // NPC) * NPC2 + (s_loc % NPC2)
    assert NCORES * NPC2 <= 32768

    # unified (block, half) group sizes = max over cores, rounded to 128
    key = (owner * NBLK + block) * 2 + half
    cnt = np.bincount(key, minlength=NCORES * NBLK * 2).reshape(NCORES, NBLK, 2)
    g_sizes = ((cnt.max(axis=0) + 127) // 128) * 128      # [NBLK, 2]
    offs = np.zeros((NBLK, 2), dtype=np.int64)
    offs.flat[1:] = np.cumsum(g_sizes.flat)[:-1]
    P = int(g_sizes.sum())
    ncht = P // 128

    # order edges by (owner, block, half); compute each edge\'s padded slot
    order = np.lexsort((half, block, owner))
    s_owner = owner[order]
    s_block = block[order]
    s_half = half[order]
    s_lidx = lidx[order]
    s_doff = dstoff[order]
    s_norm = norm[order]
    kall = s_owner * NBLK * 2 + s_block * 2 + s_half
    changes = np.empty(len(kall), dtype=bool)
    changes[0] = True
    changes[1:] = kall[1:] != kall[:-1]
    run_start = np.maximum.accumulate(np.where(changes, np.arange(len(kall)), 0))
    rank = np.arange(len(kall)) - run_start
    pos = offs[s_block, s_half] + rank   # padded position within the core

    iota = np.broadcast_to(np.arange(128, dtype=np.float32), (128, 128)).copy()

    # x table reordered to (half, owner, offset) so gathers use lidx
    v = np.arange(n, dtype=np.int64)
    tab_row = ((v % NPC) // NPC2) * (NCORES * NPC2) \
        + (v // NPC) * NPC2 + ((v % NPC) % NPC2)
    x_bf = np.empty((n, cfg.in_ch), dtype=BF16)
    x_bf[tab_row] = x.astype(BF16)

    w1_bf = W1.astype(BF16)
    w2_bf = W2.astype(BF16)
    b1_t = b1.reshape(KG, 128).T.astype(np.float32).copy()
    b2b = np.broadcast_to(b2, (128, OUT_CH)).astype(np.float32).copy()

    in_maps = []
    for c in range(NCORES):
        m = s_owner == c
        p = pos[m]
        idx_p = np.zeros(P, dtype=np.int16)      # pads gather row 0, S=0
        dof_p = np.zeros(P, dtype=np.float32)
        nrm_p = np.zeros(P, dtype=np.float32)
        idx_p[p] = s_lidx[m].astype(np.int16)
        dof_p[p] = s_doff[m].astype(np.float32)
        nrm_p[p] = s_norm[m]
        # idx layout: position q -> [16r + q%16, q//16], replicated r=0..7
        idx_l = np.tile(idx_p.reshape(P // 16, 16).T, (8, 1)).copy()
        dof_l = dof_p.reshape(ncht, 128).T
        nrm_l = nrm_p.reshape(ncht, 128).T
        meta = np.concatenate([iota, dof_l, nrm_l], axis=1).astype(np.float32)
        in_maps.append({
            "x_tab": x_bf,
            "idx_in": idx_l,
            "meta_in": meta,
            "w1_in": w1_bf,
            "w2_in": w2_bf,
            "b1_in": b1_t,
            "b2b_in": b2b,
        })

    return in_maps, tuple(int(v) for v in g_sizes.flat), ncht, P, ()


# ------------------------------------------------------------- device build

_BUILD_CACHE = {}


def _build(g_flat, ncht, P, grp_lo_flat, cfg=DEFAULT_CFG, phases="ABCDE"):
    key = (g_flat, ncht, P, grp_lo_flat, cfg, phases)
    if key in _BUILD_CACHE:
        return _BUILD_CACHE[key]
    NPC, NBLK, LAST_ROWS, NPC2 = _derived(cfg)
    NCORES = cfg.ncores
    N_NODES, IN_CH, HID, OUT_CH = cfg.n_nodes, cfg.in_ch, cfg.hid, cfg.out_ch
    KG = cfg.hid // 128
    FG = cfg.in_ch // 128
    TAB = NCORES * NPC2                     # rows per table half
    g_sizes = np.asarray(g_flat, dtype=np.int64).reshape(NBLK, 2)
    dt = mybir.dt
    nc = bacc.Bacc("TRN2", target_bir_lowering=False, debug=False,
                   enable_asserts=False, num_devices=NCORES,
                   num_swdge_queues=2)

    x_tab = nc.dram_tensor("x_tab", [N_NODES, IN_CH], dt.bfloat16,
                           kind="ExternalInput").ap()
    idx_in = nc.dram_tensor("idx_in", [128, P // 16], dt.int16,
                            kind="ExternalInput").ap()
    meta_in = nc.dram_tensor("meta_in", [128, 128 + 2 * ncht], dt.float32,
                             kind="ExternalInput").ap()
    w1_in = nc.dram_tensor("w1_in", [IN_CH, HID], dt.bfloat16,
                           kind="ExternalInput").ap()
    w2_in = nc.dram_tensor("w2_in", [HID, OUT_CH], dt.bfloat16,
                           kind="ExternalInput").ap()
    b1_in = nc.dram_tensor("b1_in", [128, KG], dt.float32,
                           kind="ExternalInput").ap()
    b2b_in = nc.dram_tensor("b2b_in", [128, OUT_CH], dt.float32,
                            kind="ExternalInput").ap()
    out_sh = nc.dram_tensor("out_shard", [NPC, OUT_CH], dt.float32,
                            kind="ExternalOutput").ap()

    agg1_d = nc.dram_tensor("agg1_d", [NBLK * 128, IN_CH], dt.bfloat16)
    h2_local = nc.dram_tensor("h2_local", [NPC, OUT_CH], dt.bfloat16)
    h2_t = [nc.dram_tensor(f"h2_t{h}", [TAB, OUT_CH], dt.bfloat16,
                           addr_space="Shared") for h in range(2)]

    ncols = NBLK * 128                      # padded node columns

    with tile.TileContext(nc) as tc, ExitStack() as ctx:
        const = ctx.enter_context(tc.tile_pool(name="const", bufs=1))
        persist = ctx.enter_context(tc.tile_pool(name="persist", bufs=1))
        msgs1_p = ctx.enter_context(tc.tile_pool(name="msgs1", bufs=2))
        msgs2_p = ctx.enter_context(tc.tile_pool(name="msgs2", bufs=2))
        s_p = ctx.enter_context(tc.tile_pool(name="sbuild", bufs=8))
        small = ctx.enter_context(tc.tile_pool(name="small", bufs=3))
        psA_p = ctx.enter_context(tc.tile_pool(name="psA", bufs=2, space="PSUM"))
        psC_p = ctx.enter_context(tc.tile_pool(name="psC", bufs=2, space="PSUM"))

        idx_t = const.tile([128, P // 16], dt.int16)
        nc.sync.dma_start(idx_t[:], idx_in)
        meta_t = const.tile([128, 128 + 2 * ncht], dt.float32)
        nc.sync.dma_start(meta_t[:], meta_in)
        w1_t = const.tile([128, FG, HID], dt.bfloat16)
        nc.sync.dma_start(w1_t[:], w1_in.rearrange("(g p) n -> p g n", p=128))
        w2_t = const.tile([128, KG, OUT_CH], dt.bfloat16)
        nc.sync.dma_start(w2_t[:], w2_in.rearrange("(g p) n -> p g n", p=128))
        b1_t = const.tile([128, KG], dt.float32)
        nc.sync.dma_start(b1_t[:], b1_in)
        b2b_t = const.tile([128, OUT_CH], dt.float32)
        nc.sync.dma_start(b2b_t[:], b2b_in)
        # bf16 iota copy (2-byte DVE mode for the S builds)
        iota_bf = const.tile([128, 128], dt.bfloat16)
        nc.vector.tensor_copy(iota_bf[:], meta_t[:, 0:128])

        _qstate = [0]

        def _next_q():
            q = _qstate[0]
            _qstate[0] = (q + 1) % 2
            return q

        def s_build(cg):
            S = s_p.tile([128, 128], dt.bfloat16, tag="S")
            nc.vector.tensor_scalar(
                out=S[:], in0=iota_bf[:],
                scalar1=meta_t[:, 128 + cg:129 + cg],
                scalar2=meta_t[:, 128 + ncht + cg:129 + ncht + cg],
                op0=mybir.AluOpType.is_equal, op1=mybir.AluOpType.mult)
            return S

        def _gather(out_ap, in_ap, c0, kw, elem):
            nc.gpsimd.dma_gather(
                out_ap=out_ap, in_ap=in_ap,
                idxs_ap=idx_t[:, c0 * 8:(c0 + kw) * 8],
                num_idxs=kw * 128, num_idxs_reg=kw * 128,
                elem_size=elem, queue_num=_next_q())

        agg1T = [persist.tile([128, ncols], dt.bfloat16, tag=f"a{j}",
                              name=f"agg1T{j}") for j in range(FG)]
        reluT = [persist.tile([128, ncols], dt.bfloat16, tag=f"r{j}",
                              name=f"reluT{j}") for j in range(KG)]

        # ---- phase A: layer-1 aggregation (node-major), spill + transpose
        cg = 0
        for b in range(NBLK):
            psA = psA_p.tile([128, IN_CH], dt.float32, tag="psA")
            nch_b = int(g_sizes[b].sum()) // 128
            ci = 0
            for h in (0, 1):
                G = int(g_sizes[b, h])
                if G == 0:
                    continue
                K = G // 128
                msgs = msgs1_p.tile([128, K, IN_CH], dt.bfloat16, tag="m1")
                src_ap = x_tab[0:TAB, :] if h == 0 else x_tab[TAB:2 * TAB, :]
                k0 = 0
                while k0 < K:
                    kw = min(SUBCALL, K - k0)
                    _gather(msgs[:, k0:k0 + kw, :], src_ap, cg + k0, kw,
                            IN_CH)
                    k0 += kw
                for k in range(K):
                    S = s_build(cg)
                    nc.tensor.matmul(psA[:], S[:], msgs[:, k, :],
                                     start=(ci == 0), stop=(ci == nch_b - 1))
                    ci += 1
                    cg += 1
            a1sb = small.tile([128, IN_CH], dt.bfloat16, tag="a1sb")
            nc.vector.tensor_copy(a1sb[:], psA[:])
            nc.sync.dma_start(agg1_d[128 * b:128 * (b + 1), :], a1sb[:])
        # feature-major operand via XBAR transpose
        for j in range(FG):
            nc.sync.dma_start_transpose(
                agg1T[j][:], agg1_d[:, 128 * j:128 * (j + 1)])

        # ---- phase B: out1T = W1^T @ agg1T (+b1, relu)  [feature-major]
        node_chunks = [(s, min(512, ncols - s)) for s in range(0, ncols, 512)]
        for j in range(KG if "B" in phases else 0):
            for (ns, nw) in node_chunks:
                psB = psA_p.tile([128, nw], dt.float32, tag="psA")
                for g in range(FG):
                    nc.tensor.matmul(psB[:], w1_t[:, g, 128 * j:128 * (j + 1)],
                                     agg1T[g][:, ns:ns + nw],
                                     start=(g == 0), stop=(g == FG - 1))
                nc.vector.tensor_scalar(
                    out=reluT[j][:, ns:ns + nw], in0=psB[:],
                    scalar1=b1_t[:, j:j + 1], scalar2=0.0,
                    op0=mybir.AluOpType.add, op1=mybir.AluOpType.max)

        # ---- phase C: h2 = reluT^T @ W2 (node-major), to DRAM for AG
        for t in range(NBLK if "C" in phases else 0):
            rows = 128 if t < NBLK - 1 else LAST_ROWS
            psC = psC_p.tile([128, OUT_CH], dt.float32, tag="psC")
            for g in range(KG):
                nc.tensor.matmul(psC[:], reluT[g][:, 128 * t:128 * (t + 1)],
                                 w2_t[:, g, :],
                                 start=(g == 0), stop=(g == KG - 1))
            h2sb = small.tile([128, OUT_CH], dt.bfloat16, tag="h2sb")
            nc.vector.tensor_copy(h2sb[:], psC[:])
            nc.sync.dma_start(h2_local[128 * t:128 * t + rows, :],
                              h2sb[:rows, :])

        # ---- phase D: AllGather h2 in two half-shard collectives
        if "D" in phases:
            for h in range(2):
                nc.gpsimd.collective_compute(
                    "AllGather", mybir.AluOpType.bypass,
                    replica_groups=[list(range(NCORES))],
                    ins=[h2_local.ap()[h * NPC2:(h + 1) * NPC2, :].opt()],
                    outs=[h2_t[h].ap().opt()])

        # ---- phase E: layer-2 aggregation (node-major) + b2 -> output
        cg = 0
        for b in range(NBLK if "E" in phases else 0):
            rows = 128 if b < NBLK - 1 else LAST_ROWS
            psE = psC_p.tile([128, OUT_CH], dt.float32, tag="psC")
            nch_b = int(g_sizes[b].sum()) // 128
            ci = 0
            for h in (0, 1):
                G = int(g_sizes[b, h])
                if G == 0:
                    continue
                K = G // 128
                msgs2 = msgs2_p.tile([128, K, OUT_CH], dt.bfloat16, tag="m2")
                src_ap = h2_t[h].ap()
                k0 = 0
                while k0 < K:
                    kw = min(SUBCALL, K - k0)
                    _gather(msgs2[:, k0:k0 + kw, :], src_ap, cg + k0, kw,
                            OUT_CH)
                    k0 += kw
                for k in range(K):
                    S = s_build(cg)
                    nc.tensor.matmul(psE[:], S[:], msgs2[:, k, :],
                                     start=(ci == 0), stop=(ci == nch_b - 1))
                    ci += 1
                    cg += 1
            outsb = small.tile([128, OUT_CH], dt.float32, tag="outsb")
            nc.vector.tensor_add(outsb[:], psE[:], b2b_t[:])
            nc.sync.dma_start(out_sh[128 * b:128 * b + rows, :],
                              outsb[:rows, :])

        if "E" not in phases:
            dummy = small.tile([128, OUT_CH], dt.float32, tag="outsb")
            nc.vector.tensor_copy(dummy[:], agg1T[0][:, 0:OUT_CH])
            nc.sync.dma_start(out_sh[0:128, :], dummy[:])

    nc.compile()
    _BUILD_CACHE[key] = nc
    return nc


# ------------------------------------------------------------------- driver

def kernel(x, edge_index, W1, b1, W2, b2, cfg=DEFAULT_CFG):
    in_maps, g_flat, ncht, P, grp_lo = _preprocess(
        x, edge_index, W1, b1, W2, b2, cfg)
    nc = _build(g_flat, ncht, P, grp_lo, cfg)
    res = run_bass_kernel_spmd(nc, in_maps, list(range(cfg.ncores)))
    out = np.concatenate(
        [res.results[c]["out_shard"] for c in range(cfg.ncores)], axis=0)
    return out.astype(np.float32)



# revision 1
# speedup vs baseline: 1.4579x; 1.4579x over previous
"""Trainium2 Bass kernel for a 2-layer GCN encoder (PyG GCNConv semantics).

Math (per gcn_conv): out = D^-1/2 (A+I) D^-1/2 (x @ W) + b, with relu
between the two convs.

Strategy (8 NeuronCores, SPMD):
  * Layer 1 is computed as (A_hat @ x) @ W1 + b1 (associativity), so the
    edge aggregation runs directly on the input x, which is replicated into
    every core's DRAM for free -> layer 1 needs no communication.
  * Nodes (aggregation outputs) are sharded by destination: core c owns
    nodes [6250c, 6250(c+1)). Edges are partitioned by dst owner and
    grouped by 128-node dst blocks.
  * Aggregation = gather + scatter-matmul: source rows are fetched with the
    GPSIMD dma_gather custom op (bf16 rows); a per-chunk selection matrix
    S[e, slot] = norm_e * (slot == dstoff_e) is built with one DVE
    tensor_scalar (iota compare), and TensorE matmuls with lhsT=S
    scatter-add 128-edge chunks into a [slot, feat] PSUM block.
  * Layer-1 aggregation lands node-major; a bf16 DMA-transpose (XBAR)
    produces the feature-major operand for the W1 GEMM. relu/bias run in
    the PSUM->SBUF epilogues. h2 = relu(out1) @ W2 stays local.
  * The only communication is an AllGather of h2, split into two
    half-shard collectives so layer-2 gathers of the first half overlap
    the second collective.
  * dma_gather indices are int16; tables are stored/addressed in two
    halves ordered by (local-half, owner, offset), so indices stay < 25088.
    Gather-call padding uses idx=-1 (descriptors skipped); per-core valid
    counts feed num_idxs_reg via a register.

Host-side work is limited to index/partition metadata (edge bucketing,
padding, degree-based norm coefficients) and dtype/layout staging.
"""
import sys
from collections import namedtuple
from contextlib import ExitStack

sys.path.insert(0, "/opt/trn_rl_repo")

import numpy as np
import ml_dtypes

import concourse.bacc as bacc
import concourse.mybir as mybir
import concourse.tile as tile
from concourse.bass_utils import run_bass_kernel_spmd

BF16 = ml_dtypes.bfloat16

Cfg = namedtuple("Cfg", "n_nodes in_ch hid out_ch ncores split")
DEFAULT_CFG = Cfg(50000, 512, 512, 256, 8, 0)

SUBCALL = 7          # max gather chunks per dma_gather call (SWDGE ring)


def _derived(cfg):
    npc = cfg.n_nodes // cfg.ncores
    nblk = (npc + 127) // 128
    last_rows = npc - 128 * (nblk - 1)
    npc2 = npc // 2
    return npc, nblk, last_rows, npc2


def _call_schedule(g_sizes, nblk):
    calls = []
    cgb = 0
    for b in range(nblk):
        for h in (0, 1):
            G = int(g_sizes[b, h])
            K = G // 128
            k0 = 0
            while k0 < K:
                kw = min(SUBCALL, K - k0)
                calls.append((cgb + k0, kw))
                k0 += kw
            cgb += K
    return calls


# ----------------------------------------------------------------- host prep

def _preprocess(x, edge_index, W1, b1, W2, b2, cfg=DEFAULT_CFG):
    x = np.asarray(x, dtype=np.float32)
    ei = np.asarray(edge_index)
    W1 = np.asarray(W1, dtype=np.float32)
    b1 = np.asarray(b1, dtype=np.float32)
    W2 = np.asarray(W2, dtype=np.float32)
    b2 = np.asarray(b2, dtype=np.float32)

    NPC, NBLK, LAST_ROWS, NPC2 = _derived(cfg)
    NCORES = cfg.ncores
    KG = cfg.hid // 128
    OUT_CH = cfg.out_ch
    n = x.shape[0]
    loops = np.arange(n, dtype=np.int64)
    src = np.concatenate([ei[0].astype(np.int64), loops])
    dst = np.concatenate([ei[1].astype(np.int64), loops])

    # degree (with self loops) and symmetric normalization
    deg = np.bincount(dst, minlength=n).astype(np.float32)
    dinv = np.where(deg > 0, 1.0 / np.sqrt(deg), 0.0).astype(np.float32)
    norm = dinv[src] * dinv[dst]

    owner = dst // NPC
    block = (dst % NPC) // 128
    dstoff = (dst % NPC) % 128
    # source table coordinates: (half, owner, offset) ordering
    s_loc = src % NPC
    half = (s_loc >= NPC2).astype(np.int64)
    lidx = (src // NPC) * NPC2 + (s_loc % NPC2)
    assert NCORES * NPC2 <= 32768

    # unified (block, half) group sizes = max over cores, rounded to 128
    key = (owner * NBLK + block) * 2 + half
    cnt = np.bincount(key, minlength=NCORES * NBLK * 2).reshape(NCORES, NBLK, 2)
    g_sizes = ((cnt.max(axis=0) + 127) // 128) * 128      # [NBLK, 2]
    offs = np.zeros((NBLK, 2), dtype=np.int64)
    offs.flat[1:] = np.cumsum(g_sizes.flat)[:-1]
    P = int(g_sizes.sum())
    ncht = P // 128

    # order edges by (owner, block, half); compute each edge\'s padded slot
    order = np.lexsort((half, block, owner))
    s_owner = owner[order]
    s_block = block[order]
    s_half = half[order]
    s_lidx = lidx[order]
    s_doff = dstoff[order]
    s_norm = norm[order]
    kall = s_owner * NBLK * 2 + s_block * 2 + s_half
    changes = np.empty(len(kall), dtype=bool)
    changes[0] = True
    changes[1:] = kall[1:] != kall[:-1]
    run_start = np.maximum.accumulate(np.where(changes, np.arange(len(kall)), 0))
    rank = np.arange(len(kall)) - run_start
    pos = offs[s_block, s_half] + rank   # padded position within the core

    iota = np.broadcast_to(np.arange(128, dtype=np.float32), (128, 128)).copy()

    # x table reordered to (half, owner, offset) so gathers use lidx
    v = np.arange(n, dtype=np.int64)
    tab_row = ((v % NPC) // NPC2) * (NCORES * NPC2) \
        + (v // NPC) * NPC2 + ((v % NPC) % NPC2)
    x_bf = np.empty((n, cfg.in_ch), dtype=BF16)
    x_bf[tab_row] = x.astype(BF16)

    w1_bf = W1.astype(BF16)
    w2_bf = W2.astype(BF16)
    b1_t = b1.reshape(KG, 128).T.astype(np.float32).copy()
    b2b = np.broadcast_to(b2, (128, OUT_CH)).astype(np.float32).copy()

    in_maps = []
    for c in range(NCORES):
        m = s_owner == c
        p = pos[m]
        idx_p = np.zeros(P, dtype=np.int16)      # pads gather row 0, S=0
        dof_p = np.zeros(P, dtype=np.float32)
        nrm_p = np.zeros(P, dtype=np.float32)
        idx_p[p] = s_lidx[m].astype(np.int16)
        dof_p[p] = s_doff[m].astype(np.float32)
        nrm_p[p] = s_norm[m]
        # idx layout: position q -> [16r + q%16, q//16], replicated r=0..7
        idx_l = np.tile(idx_p.reshape(P // 16, 16).T, (8, 1)).copy()
        dof_l = dof_p.reshape(ncht, 128).T
        nrm_l = nrm_p.reshape(ncht, 128).T
        meta = np.concatenate([iota, dof_l, nrm_l], axis=1).astype(np.float32)
        in_maps.append({
            "x_tab": x_bf,
            "idx_in": idx_l,
            "meta_in": meta,
            "w1_in": w1_bf,
            "w2_in": w2_bf,
            "b1_in": b1_t,
            "b2b_in": b2b,
        })

    return in_maps, tuple(int(v) for v in g_sizes.flat), ncht, P, ()


# ------------------------------------------------------------- device build

_BUILD_CACHE = {}


def _build(g_flat, ncht, P, grp_lo_flat, cfg=DEFAULT_CFG, phases="ABCDE"):
    key = (g_flat, ncht, P, grp_lo_flat, cfg, phases)
    if key in _BUILD_CACHE:
        return _BUILD_CACHE[key]
    NPC, NBLK, LAST_ROWS, NPC2 = _derived(cfg)
    NCORES = cfg.ncores
    N_NODES, IN_CH, HID, OUT_CH = cfg.n_nodes, cfg.in_ch, cfg.hid, cfg.out_ch
    KG = cfg.hid // 128
    FG = cfg.in_ch // 128
    TAB = NCORES * NPC2                     # rows per table half
    g_sizes = np.asarray(g_flat, dtype=np.int64).reshape(NBLK, 2)
    dt = mybir.dt
    nc = bacc.Bacc("TRN2", target_bir_lowering=False, debug=False,
                   enable_asserts=False, num_devices=NCORES,
                   num_swdge_queues=2)

    x_tab = nc.dram_tensor("x_tab", [N_NODES, IN_CH], dt.bfloat16,
                           kind="ExternalInput").ap()
    idx_in = nc.dram_tensor("idx_in", [128, P // 16], dt.int16,
                            kind="ExternalInput").ap()
    meta_in = nc.dram_tensor("meta_in", [128, 128 + 2 * ncht], dt.float32,
                             kind="ExternalInput").ap()
    w1_in = nc.dram_tensor("w1_in", [IN_CH, HID], dt.bfloat16,
                           kind="ExternalInput").ap()
    w2_in = nc.dram_tensor("w2_in", [HID, OUT_CH], dt.bfloat16,
                           kind="ExternalInput").ap()
    b1_in = nc.dram_tensor("b1_in", [128, KG], dt.float32,
                           kind="ExternalInput").ap()
    b2b_in = nc.dram_tensor("b2b_in", [128, OUT_CH], dt.float32,
                            kind="ExternalInput").ap()
    out_sh = nc.dram_tensor("out_shard", [NPC, OUT_CH], dt.float32,
                            kind="ExternalOutput").ap()

    agg1_d = nc.dram_tensor("agg1_d", [NBLK * 128, IN_CH], dt.bfloat16)
    h2_local = nc.dram_tensor("h2_local", [NPC, OUT_CH], dt.bfloat16)
    h2_t = [nc.dram_tensor(f"h2_t{h}", [TAB, OUT_CH], dt.bfloat16,
                           addr_space="Shared") for h in range(2)]

    ncols = NBLK * 128                      # padded node columns

    with tile.TileContext(nc) as tc, ExitStack() as ctx:
        const = ctx.enter_context(tc.tile_pool(name="const", bufs=1))
        persist = ctx.enter_context(tc.tile_pool(name="persist", bufs=1))
        msgs1_p = ctx.enter_context(tc.tile_pool(name="msgs1", bufs=2))
        msgs2_p = ctx.enter_context(tc.tile_pool(name="msgs2", bufs=2))
        s_p = ctx.enter_context(tc.tile_pool(name="sbuild", bufs=8))
        small = ctx.enter_context(tc.tile_pool(name="small", bufs=3))
        psA_p = ctx.enter_context(tc.tile_pool(name="psA", bufs=2, space="PSUM"))
        psC_p = ctx.enter_context(tc.tile_pool(name="psC", bufs=2, space="PSUM"))

        idx_t = const.tile([128, P // 16], dt.int16)
        nc.sync.dma_start(idx_t[:], idx_in)
        meta_t = const.tile([128, 128 + 2 * ncht], dt.float32)
        nc.sync.dma_start(meta_t[:], meta_in)
        w1_t = const.tile([128, FG, HID], dt.bfloat16)
        nc.sync.dma_start(w1_t[:], w1_in.rearrange("(g p) n -> p g n", p=128))
        w2_t = const.tile([128, KG, OUT_CH], dt.bfloat16)
        nc.sync.dma_start(w2_t[:], w2_in.rearrange("(g p) n -> p g n", p=128))
        b1_t = const.tile([128, KG], dt.float32)
        nc.sync.dma_start(b1_t[:], b1_in)
        b2b_t = const.tile([128, OUT_CH], dt.float32)
        nc.sync.dma_start(b2b_t[:], b2b_in)
        # bf16 iota copy (2-byte DVE mode for the S builds)
        iota_bf = const.tile([128, 128], dt.bfloat16)
        nc.vector.tensor_copy(iota_bf[:], meta_t[:, 0:128])

        _qstate = [0]

        def _next_q():
            q = _qstate[0]
            _qstate[0] = (q + 1) % 2
            return q

        def s_build(cg):
            S = s_p.tile([128, 128], dt.bfloat16, tag="S")
            nc.vector.tensor_scalar(
                out=S[:], in0=iota_bf[:],
                scalar1=meta_t[:, 128 + cg:129 + cg],
                scalar2=meta_t[:, 128 + ncht + cg:129 + ncht + cg],
                op0=mybir.AluOpType.is_equal, op1=mybir.AluOpType.mult)
            return S

        def _gather(out_ap, in_ap, c0, kw, elem):
            nc.gpsimd.dma_gather(
                out_ap=out_ap, in_ap=in_ap,
                idxs_ap=idx_t[:, c0 * 8:(c0 + kw) * 8],
                num_idxs=kw * 128, num_idxs_reg=kw * 128,
                elem_size=elem, queue_num=_next_q())

        agg1T = [persist.tile([128, ncols], dt.bfloat16, tag=f"a{j}",
                              name=f"agg1T{j}") for j in range(FG)]
        reluT = [persist.tile([128, ncols], dt.bfloat16, tag=f"r{j}",
                              name=f"reluT{j}") for j in range(KG)]

        # ---- phase A: layer-1 aggregation (node-major), spill + transpose
        cg = 0
        for b in range(NBLK):
            psA = psA_p.tile([128, IN_CH], dt.float32, tag="psA")
            nch_b = int(g_sizes[b].sum()) // 128
            ci = 0
            for h in (0, 1):
                G = int(g_sizes[b, h])
                if G == 0:
                    continue
                K = G // 128
                msgs = msgs1_p.tile([128, K, IN_CH], dt.bfloat16, tag="m1")
                src_ap = x_tab[0:TAB, :] if h == 0 else x_tab[TAB:2 * TAB, :]
                k0 = 0
                while k0 < K:
                    kw = min(SUBCALL, K - k0)
                    _gather(msgs[:, k0:k0 + kw, :], src_ap, cg + k0, kw,
                            IN_CH)
                    k0 += kw
                for k in range(K):
                    S = s_build(cg)
                    nc.tensor.matmul(psA[:], S[:], msgs[:, k, :],
                                     start=(ci == 0), stop=(ci == nch_b - 1))
                    ci += 1
                    cg += 1
            a1sb = small.tile([128, IN_CH], dt.bfloat16, tag="a1sb")
            nc.vector.tensor_copy(a1sb[:], psA[:])
            nc.sync.dma_start(agg1_d[128 * b:128 * (b + 1), :], a1sb[:])
        # feature-major operand via XBAR transpose
        for j in range(FG):
            nc.sync.dma_start_transpose(
                agg1T[j][:], agg1_d[:, 128 * j:128 * (j + 1)])

        # ---- phase B: out1T = W1^T @ agg1T (+b1, relu)  [feature-major]
        node_chunks = [(s, min(512, ncols - s)) for s in range(0, ncols, 512)]
        for j in range(KG if "B" in phases else 0):
            for (ns, nw) in node_chunks:
                psB = psA_p.tile([128, nw], dt.float32, tag="psA")
                for g in range(FG):
                    nc.tensor.matmul(psB[:], w1_t[:, g, 128 * j:128 * (j + 1)],
                                     agg1T[g][:, ns:ns + nw],
                                     start=(g == 0), stop=(g == FG - 1))
                nc.vector.tensor_scalar(
                    out=reluT[j][:, ns:ns + nw], in0=psB[:],
                    scalar1=b1_t[:, j:j + 1], scalar2=0.0,
                    op0=mybir.AluOpType.add, op1=mybir.AluOpType.max)

        # ---- phase C: h2 = reluT^T @ W2 (node-major), to DRAM for AG
        for t in range(NBLK if "C" in phases else 0):
            rows = 128 if t < NBLK - 1 else LAST_ROWS
            psC = psC_p.tile([128, OUT_CH], dt.float32, tag="psC")
            for g in range(KG):
                nc.tensor.matmul(psC[:], reluT[g][:, 128 * t:128 * (t + 1)],
                                 w2_t[:, g, :],
                                 start=(g == 0), stop=(g == KG - 1))
            h2sb = small.tile([128, OUT_CH], dt.bfloat16, tag="h2sb")
            nc.vector.tensor_copy(h2sb[:], psC[:])
            nc.sync.dma_start(h2_local[128 * t:128 * t + rows, :],
                              h2sb[:rows, :])

        # ---- phase D: AllGather h2 in two half-shard collectives
        if "D" in phases:
            for h in range(2):
                nc.gpsimd.collective_compute(
                    "AllGather", mybir.AluOpType.bypass,
                    replica_groups=[list(range(NCORES))],
                    ins=[h2_local.ap()[h * NPC2:(h + 1) * NPC2, :].opt()],
                    outs=[h2_t[h].ap().opt()])

        # ---- phase E: layer-2 aggregation (node-major) + b2 -> output
        cg = 0
        for b in range(NBLK if "E" in phases else 0):
            rows = 128 if b < NBLK - 1 else LAST_ROWS
            psE = psC_p.tile([128, OUT_CH], dt.float32, tag="psC")
            nch_b = int(g_sizes[b].sum()) // 128
            ci = 0
            for h in (0, 1):
                G = int(g_sizes[b, h])
                if G == 0:
                    continue
                K = G // 128
                msgs2 = msgs2_p.tile([128, K, OUT_CH], dt.bfloat16, tag="m2")
                src_ap = h2_t[h].ap()
                k0 = 0
                while k0 < K:
                    kw = min(SUBCALL, K - k0)
                    _gather(msgs2[:, k0:k0 + kw, :], src_ap, cg + k0, kw,
                            OUT_CH)
                    k0 += kw
                for k in range(K):
                    S = s_build(cg)
                    nc.tensor.matmul(psE[:], S[:], msgs2[:, k, :],
                                     start=(ci == 0), stop=(ci == nch_b - 1))
                    ci += 1
                    cg += 1
            outsb = small.tile([128, OUT_CH], dt.float32, tag="outsb")
            nc.vector.tensor_add(outsb[:], psE[:], b2b_t[:])
            nc.sync.dma_start(out_sh[128 * b:128 * b + rows, :],
                              outsb[:rows, :])

        if "E" not in phases:
            dummy = small.tile([128, OUT_CH], dt.float32, tag="outsb")
            nc.vector.tensor_copy(dummy[:], agg1T[0][:, 0:OUT_CH])
            nc.sync.dma_start(out_sh[0:128, :], dummy[:])

    nc.compile()
    _BUILD_CACHE[key] = nc
    return nc


# ------------------------------------------------------------------- driver

def kernel(x, edge_index, W1, b1, W2, b2, cfg=DEFAULT_CFG):
    in_maps, g_flat, ncht, P, grp_lo = _preprocess(
        x, edge_index, W1, b1, W2, b2, cfg)
    nc = _build(g_flat, ncht, P, grp_lo, cfg)
    res = run_bass_kernel_spmd(nc, in_maps, list(range(cfg.ncores)))
    out = np.concatenate(
        [res.results[c]["out_shard"] for c in range(cfg.ncores)], axis=0)
    return out.astype(np.float32)

